# revision 1
# baseline (speedup 1.0000x reference)
"""Trainium2 Bass kernel for nn_KabschDecoder: per-box sigmoid point weights.

Computes w[b,s,n] = sig(7*(hx-|x'|)) * sig(7*(hy-|y'|)) * sig(7*(hz-|z'|))
where (x',y',z') is lidar point n expressed in box (b,s)'s frame (SE(3),
rotation about z only), and h* are box half-dims.

Strategy (8 NeuronCores, SPMD, no collectives):
  - Shard the N (points) axis 8 ways: each core handles all 256 boxes for
    its 8192-point slice. Host gathers along N.
  - Host precomputes, per box, the 3 rows of 7*inv(s_T_box) (tiny: 256x12
    floats) and 7*dims/2. These feed the TensorEngine as weights.
  - Device, per core: PE (float32r matmuls, K=8 block-diagonal packing of
    2 batches x 64 boxes = 128 output rows) produces v_c = 7*x'_c in PSUM;
    DVE tensor_reduce(apply_absolute_value, dummy axis) drains PSUM to
    fp16 |v|; ACT evaluates sig(h7 - |v|) via per-partition bias; DVE/
    GPSIMD multiply the three factors; DMA writes f32 rows to HBM.
"""

import sys

sys.path.insert(0, "/opt/trn_rl_repo")

import numpy as np

import concourse.bass as bass
import concourse.tile as tile
from concourse import mybir
from concourse.bass_utils import run_bass_kernel_spmd

B, S, N = 4, 64, 65536
NCORES = 8
NSH = N // NCORES          # 8192 points per core
FD = 2048                  # free-dim chunk (4 PSUM banks)
NPAIR = B // 2             # batches packed per 128-row group
SIGMOID_SLOPE = 7.0
HALF = 0.5                 # OBJ_DIM_SCALE * 0.5

F32 = mybir.dt.float32
F32R = mybir.dt.float32r
F16 = mybir.dt.float16


MAX_WAITS_PER_INST = 1


def _split_sync_waits(nc: bass.Bass, limit: int = MAX_WAITS_PER_INST):
    """This walrus build rejects instructions carrying more than ~1 sync
    wait command. Move excess waits onto same-engine NOPs inserted just
    before the over-subscribed instruction (engines execute their queue in
    order, so this is semantically identical)."""
    uid = 0
    for fn in nc.m.functions:
        for blk in fn.blocks:
            insts = list(blk.instructions)
            out = []
            changed = False
            for ins in insts:
                si = ins.sync_info
                if si is not None and si.on_wait and len(si.on_wait) > limit:
                    waits = list(si.on_wait)
                    keep = waits[:limit]
                    rest = waits[limit:]
                    ins.sync_info = mybir.SyncInfo(
                        on_wait=keep, on_update=list(si.on_update)
                    )
                    for i in range(0, len(rest), limit):
                        nop = mybir.InstNoOp(
                            name=f"waitsplit-{uid}",
                            ins=[],
                            outs=[],
                            engine=ins.engine,
                        )
                        nop.sync_info = mybir.SyncInfo(
                            on_wait=list(rest[i : i + limit]), on_update=[]
                        )
                        uid += 1
                        out.append(nop)
                    changed = True
                out.append(ins)
            if changed:
                blk.instructions = out


def _build_nc() -> bass.Bass:
    nc = bass.Bass("TRN2", target_bir_lowering=False, debug=False)
    rhs_d = nc.dram_tensor("rhs", [NPAIR, 8, NSH], F32R, kind="ExternalInput").ap()
    wmat_d = nc.dram_tensor("wmat", [NPAIR, 3, 8, 128], F32R, kind="ExternalInput").ap()
    hvec_d = nc.dram_tensor("hvec", [NPAIR, 3, 128], F32, kind="ExternalInput").ap()
    out_d = nc.dram_tensor("out", [2 * S * NPAIR, NSH], F32, kind="ExternalOutput").ap()

    with tile.TileContext(nc) as tc:
        with (
            tc.tile_pool(name="const", bufs=1) as cpool,
            tc.tile_pool(name="psum", bufs=2, space="PSUM") as ppool,
            tc.tile_pool(name="sig", bufs=3) as spool,
            tc.tile_pool(name="fin", bufs=3) as fpool,
        ):
            rhs_sb = []
            w_sb = []
            h_sb = []
            for g in range(NPAIR):
                r = cpool.tile([8, NSH], F32R, tag=f"rhs{g}")
                nc.gpsimd.dma_start(r[:], rhs_d[g])
                rhs_sb.append(r)
                wg, hg = [], []
                for c in range(3):
                    w = cpool.tile([8, 128], F32R, tag=f"w{g}{c}")
                    nc.gpsimd.dma_start(w[:], wmat_d[g, c])
                    wg.append(w)
                    h = cpool.tile([128, 1], F32, tag=f"h{g}{c}")
                    nc.gpsimd.dma_start(h[:], hvec_d[g, c].rearrange("(m one) -> m one", one=1))
                    hg.append(h)
                w_sb.append(wg)
                h_sb.append(hg)

            nj = NSH // FD
            for g in range(NPAIR):
                for j in range(nj):
                    wsig = []
                    for c in range(3):
                        v = ppool.tile([128, FD], F32, tag="v")
                        for q in range(FD // 512):
                            col = j * FD + q * 512
                            nc.tensor.matmul(
                                v[:, q * 512 : (q + 1) * 512],
                                w_sb[g][c][:],
                                rhs_sb[g][:, col : col + 512],
                                start=True,
                                stop=True,
                            )
                        t = spool.tile([128, FD], F32, tag="t")
                        nc.vector.tensor_reduce(
                            t[:],
                            v[:].rearrange("p (f one) -> p f one", one=1),
                            axis=mybir.AxisListType.X,
                            op=mybir.AluOpType.max,
                            apply_absolute_value=True,
                        )
                        ws = spool.tile([128, FD], F16, tag=f"ws{c}")
                        nc.scalar.activation(
                            ws[:],
                            t[:],
                            mybir.ActivationFunctionType.Sigmoid,
                            bias=h_sb[g][c][:],
                            scale=-SIGMOID_SLOPE,
                        )
                        wsig.append(ws)
                    wxy = spool.tile([128, FD], F16, tag="wxy")
                    nc.vector.tensor_tensor(
                        wxy[:], wsig[0][:], wsig[1][:], op=mybir.AluOpType.mult
                    )
                    wfin = fpool.tile([128, FD], F32, tag="wfin")
                    nc.vector.tensor_tensor(
                        wfin[:], wxy[:], wsig[2][:], op=mybir.AluOpType.mult
                    )
                    nc.sync.dma_start(
                        out_d[g * 128 : (g + 1) * 128, j * FD : (j + 1) * FD],
                        wfin[:],
                    )
    _split_sync_waits(nc)
    return nc


_NC_CACHE = None


def _get_nc():
    global _NC_CACHE
    if _NC_CACHE is None:
        _NC_CACHE = _build_nc()
    return _NC_CACHE


def _host_prep(pos, dims, rot, points, valid_mask):
    pos = np.asarray(pos, dtype=np.float32)
    dims = np.asarray(dims, dtype=np.float32)
    rot = np.asarray(rot, dtype=np.float32)
    points = np.asarray(points, dtype=np.float32)
    valid_mask = np.asarray(valid_mask)

    pts = np.where(valid_mask[..., None], points, np.float32(0.0))  # (B,N,3)

    c = np.cos(rot[..., 0])  # (B,S)
    s = np.sin(rot[..., 0])
    tx, ty, tz = pos[..., 0], pos[..., 1], pos[..., 2]
    zero = np.zeros_like(c)
    one = np.ones_like(c)
    # rows of inv(s_T_box) (top 3 rows), scaled by SIGMOID_SLOPE
    rows = np.stack(
        [
            np.stack([c, s, zero, -(c * tx + s * ty)], axis=-1),
            np.stack([-s, c, zero, s * tx - c * ty], axis=-1),
            np.stack([zero, zero, one, -tz], axis=-1),
        ],
        axis=-2,
    )  # (B, S, 3, 4)
    rows = rows.astype(np.float32)

    # Block-diagonal PE weights: wmat[g, c, k, m], m = 64*half + s_box
    wmat = np.zeros((NPAIR, 3, 8, 128), dtype=np.float32)
    for g in range(NPAIR):
        for half in range(2):
            b = 2 * g + half
            # rows[b] : (S, 3, 4) -> weights k=4*half..4*half+3, m=64*half..+S
            wmat[g, :, 4 * half : 4 * half + 4, 64 * half : 64 * half + S] = (
                rows[b].transpose(1, 2, 0)
            )

    hvec = np.zeros((NPAIR, 3, 128), dtype=np.float32)
    harr = (SIGMOID_SLOPE * HALF * dims).astype(np.float32)  # (B,S,3)
    for g in range(NPAIR):
        for half in range(2):
            b = 2 * g + half
            hvec[g, :, 64 * half : 64 * half + S] = harr[b].T

    # rhs[g, k, n]: homogeneous points of the two batches stacked along K
    rhs = np.zeros((NPAIR, 8, N), dtype=np.float32)
    for g in range(NPAIR):
        for half in range(2):
            b = 2 * g + half
            rhs[g, 4 * half : 4 * half + 3] = pts[b].T
            rhs[g, 4 * half + 3] = 1.0
    return rhs, wmat, hvec


def kernel(pos, dims, rot, points, valid_mask, _want_trace=False):
    rhs, wmat, hvec = _host_prep(pos, dims, rot, points, valid_mask)

    in_maps = []
    for core in range(NCORES):
        n0 = core * NSH
        in_maps.append(
            {
                "rhs": np.ascontiguousarray(rhs[:, :, n0 : n0 + NSH]),
                "wmat": wmat,
                "hvec": hvec,
            }
        )

    nc = _get_nc()
    res = run_bass_kernel_spmd(
        nc, in_maps, core_ids=list(range(NCORES)), trace=_want_trace
    )

    out = np.empty((B * S, N), dtype=np.float32)
    for core in range(NCORES):
        n0 = core * NSH
        out[:, n0 : n0 + NSH] = res.results[core]["out"]
    result = out.reshape(B, S, N)
    if _want_trace:
        return result, res
    return result



# revision 2
# speedup vs baseline: 1.1352x; 1.1352x over previous
"""Trainium2 Bass kernel for nn_KabschDecoder: per-box sigmoid point weights.

Computes w[b,s,n] = sig(7*(hx-|x'|)) * sig(7*(hy-|y'|)) * sig(7*(hz-|z'|))
where (x',y',z') is lidar point n expressed in box (b,s)'s frame (SE(3),
rotation about z only), and h* are box half-dims.

v4 (8 NeuronCores, SPMD), per core 2.1M outputs:
  - N sharded 8 ways. x',y' via K=6 block-diagonal f32r matmuls (2 batches
    x 64 boxes = 128 PSUM partitions), 512-col chunks, [128,2048] PSUM
    tiles. Their f32 PSUM drains (16 x 2048) split DVE tensor_reduce(abs)
    / ACT Abs (same LUT table as Sigmoid -> no reload).
  - z' skips the PE entirely: host stages pz pre-broadcast to 128 rows;
    one fused DVE tensor_scalar (pz - tz, |.|) per 4096 slab runs in
    2x_2p mode (all-SBUF f32 = 2 elem/cycle).
  - sigmoids (bias 7*dims/2, scale -7) wide [128,4096] f16 on ACT.
  - the two product multiplies as scalar_tensor_tensor (4x_2p: f16
    all-SBUF = 4 elem/cycle) on DVE.
  - f16 output, host upcasts. Consts packed into 2 DMAs; points loaded
    in 2048-col chunks across the sync/gpsimd queues so the first matmul
    starts early.
"""

import sys

sys.path.insert(0, "/opt/trn_rl_repo")

import numpy as np

import concourse.bass as bass
import concourse.tile as tile
from concourse import mybir
from concourse.bass_utils import run_bass_kernel_spmd

B, S, N = 4, 64, 65536
NCORES = 8
NSH = N // NCORES          # 8192 points per core
FD = 2048                  # PSUM tile free dim (4 banks)
WIDE = 4096                # sigmoid / multiply / z instruction width
NPAIR = B // 2             # batches packed per 128-row group
SIGMOID_SLOPE = 7.0
HALF = 0.5                 # OBJ_DIM_SCALE * 0.5
ACT_DRAIN_IDX = {1, 4, 7, 10, 13}  # x/y drains routed to ACT (Abs)

F32 = mybir.dt.float32
F32R = mybir.dt.float32r
F16 = mybir.dt.float16


MAX_WAITS_PER_INST = 1


def _split_sync_waits(nc: bass.Bass, limit: int = MAX_WAITS_PER_INST):
    """This walrus build rejects instructions carrying more than ~1 sync
    wait command. Move excess waits onto same-engine NOPs inserted just
    before the over-subscribed instruction (engines execute their queue in
    order, so this is semantically identical)."""
    uid = 0
    for fn in nc.m.functions:
        for blk in fn.blocks:
            insts = list(blk.instructions)
            out = []
            changed = False
            for ins in insts:
                si = ins.sync_info
                if si is not None and si.on_wait and len(si.on_wait) > limit:
                    waits = list(si.on_wait)
                    keep = waits[:limit]
                    rest = waits[limit:]
                    ins.sync_info = mybir.SyncInfo(
                        on_wait=keep, on_update=list(si.on_update)
                    )
                    for i in range(0, len(rest), limit):
                        nop = mybir.InstNoOp(
                            name=f"waitsplit-{uid}",
                            ins=[],
                            outs=[],
                            engine=ins.engine,
                        )
                        nop.sync_info = mybir.SyncInfo(
                            on_wait=list(rest[i : i + limit]), on_update=[]
                        )
                        uid += 1
                        out.append(nop)
                    changed = True
                out.append(ins)
            if changed:
                blk.instructions = out


def _build_nc() -> bass.Bass:
    nc = bass.Bass("TRN2", target_bir_lowering=False, debug=False)
    rhs_d = nc.dram_tensor("rhs", [NPAIR, 6, NSH], F32R, kind="ExternalInput").ap()
    # wall: 6 weight mats [6,128] packed side by side (g,c) -> col block
    wall_d = nc.dram_tensor("wall", [6, NPAIR * 2 * 128], F32R, kind="ExternalInput").ap()
    # hall: sigmoid biases 7*dims/2 per (g,c); tzv appended as cols 6..7
    hall_d = nc.dram_tensor("hall", [128, NPAIR * 3 + NPAIR], F32, kind="ExternalInput").ap()
    pzb_d = nc.dram_tensor("pzb", [NPAIR, 128, NSH], F32, kind="ExternalInput").ap()
    out_d = nc.dram_tensor("out", [2 * S * NPAIR, NSH], F16, kind="ExternalOutput").ap()

    nwide = NSH // WIDE      # 2 wide slabs per group
    per_wide = WIDE // FD    # 2 PSUM tiles per slab

    with tile.TileContext(nc) as tc:
        with (
            tc.tile_pool(name="const", bufs=1) as cpool,
            tc.tile_pool(name="psum", bufs=2, space="PSUM") as ppool,
            tc.tile_pool(name="pzs", bufs=2) as zpool,
            tc.tile_pool(name="tband", bufs=2) as tpool,
            tc.tile_pool(name="sig", bufs=2) as spool,
            tc.tile_pool(name="fin", bufs=2) as fpool,
        ):
            wall = cpool.tile([6, NPAIR * 2 * 128], F32R, tag="wall")
            nc.gpsimd.dma_start(wall[:], wall_d)
            hall = cpool.tile([128, NPAIR * 3 + NPAIR], F32, tag="hall")
            nc.gpsimd.dma_start(hall[:], hall_d)

            def wslice(g, c):  # stationary [6, 128] for matmul
                o = (g * 2 + c) * 128
                return wall[:, o : o + 128]

            def hslice(g, c):  # per-partition sigmoid bias [128, 1]
                return hall[:, g * 3 + c : g * 3 + c + 1]

            def tzslice(g):  # per-partition tz [128, 1]
                return hall[:, NPAIR * 3 + g : NPAIR * 3 + g + 1]

            # Point loads: half-group [6,4096] tiles rotated through 2 slots
            # (g1 reuses g0's SBUF after g0's matmuls consume it). Queue
            # discipline: sync carries only the small first chunk + output
            # DMAs (no long slot-waits may block it); scalar takes the rest
            # of g0; gpsimd takes pz and g1 in consumption order.
            rhs_half = {}
            pz_tiles = {}

            def rhs_load(g, hw, widths, qs):
                r = tpool.tile([6, WIDE], F32R, tag="rh", name=f"rh{g}{hw}")
                o = 0
                for ch, width in enumerate(widths):
                    sl = slice(o, o + width)
                    gsl = slice(hw * WIDE + o, hw * WIDE + o + width)
                    o += width
                    qs[ch].dma_start(r[:, sl], rhs_d[g][:, gsl])
                rhs_half[(g, hw)] = r

            def pz_load(g, jw):
                pzt = zpool.tile([128, WIDE], F32, tag="pz", name=f"pz{g}{jw}")
                nc.gpsimd.dma_start(
                    pzt[:], pzb_d[g][:, jw * WIDE : (jw + 1) * WIDE]
                )
                pz_tiles[(g, jw)] = pzt

            rhs_load(0, 0, [1024, 3072], [nc.sync, nc.sync])
            rhs_load(0, 1, [2048, 2048], [nc.sync, nc.sync])
            pz_load(0, 0)
            pz_load(0, 1)
            rhs_load(1, 0, [2048, 2048], [nc.gpsimd, nc.gpsimd])
            pz_load(1, 0)
            pz_load(1, 1)
            rhs_load(1, 1, [2048, 2048], [nc.gpsimd, nc.gpsimd])

            drain_i = 0
            for g in range(NPAIR):
                for jw in range(nwide):
                    tband = []
                    for c in range(2):
                        t = tpool.tile([128, WIDE], F16, tag=f"t{c}")
                        tband.append(t)
                    # z channel: z' = pz - tz (f32 SBUF -> f16, 2x_2p),
                    # then |z'| by clearing the f16 sign bit (u16 AND, 4x)
                    tz = tpool.tile([128, WIDE], F16, tag="t2")
                    zr = fpool.tile([128, WIDE], F16, tag="zr", bufs=1)
                    nc.vector.tensor_scalar(
                        zr[:],
                        pz_tiles[(g, jw)][:],
                        tzslice(g),
                        None,
                        op0=mybir.AluOpType.subtract,
                    )
                    nc.vector.tensor_scalar(
                        tz[:].bitcast(mybir.dt.uint16),
                        zr[:].bitcast(mybir.dt.uint16),
                        0x7FFF,
                        None,
                        op0=mybir.AluOpType.bitwise_and,
                    )
                    for jj in range(per_wide):
                        j = jw * per_wide + jj
                        for c in range(2):
                            v = ppool.tile([128, FD], F32, tag="v")
                            for q4 in range(FD // 512):
                                col = jj * FD + q4 * 512
                                nc.tensor.matmul(
                                    v[:, q4 * 512 : (q4 + 1) * 512],
                                    wslice(g, c),
                                    rhs_half[(g, jw)][:, col : col + 512],
                                    start=True,
                                    stop=True,
                                )
                            tsl = tband[c][:, jj * FD : (jj + 1) * FD]
                            use_act = drain_i in ACT_DRAIN_IDX
                            drain_i += 1
                            if use_act:
                                nc.scalar.activation(
                                    tsl, v[:], mybir.ActivationFunctionType.Abs
                                )
                            else:
                                nc.vector.tensor_reduce(
                                    tsl,
                                    v[:].rearrange("p (f one) -> p f one", one=1),
                                    axis=mybir.AxisListType.X,
                                    op=mybir.AluOpType.max,
                                    apply_absolute_value=True,
                                )
                    last = (g, jw) == (NPAIR - 1, nwide - 1)
                    wsig = [None, None, None]
                    for c in (2, 0, 1):
                        src = tband[c] if c < 2 else tz
                        ws = spool.tile([128, WIDE], F16, tag=f"ws{c}", name=f"ws{c}_{g}_{jw}")
                        if c == 2 or not last:
                            nc.scalar.activation(
                                ws[:],
                                src[:],
                                mybir.ActivationFunctionType.Sigmoid,
                                bias=hslice(g, c),
                                scale=-SIGMOID_SLOPE,
                            )
                        else:
                            for jj in range(per_wide):
                                fsl = slice(jj * FD, (jj + 1) * FD)
                                nc.scalar.activation(
                                    ws[:, fsl],
                                    src[:, fsl],
                                    mybir.ActivationFunctionType.Sigmoid,
                                    bias=hslice(g, c),
                                    scale=-SIGMOID_SLOPE,
                                )
                        wsig[c] = ws
                    wxy = spool.tile([128, WIDE], F16, tag="wxy")
                    wfin = fpool.tile([128, WIDE], F16, tag="wfin")
                    parts = [slice(0, WIDE)] if not last else [
                        slice(jj * FD, (jj + 1) * FD) for jj in range(per_wide)
                    ]
                    for fsl in parts:
                        nc.vector.tensor_tensor(
                            wxy[:, fsl],
                            wsig[0][:, fsl],
                            wsig[1][:, fsl],
                            op=mybir.AluOpType.mult,
                        )
                        nc.vector.tensor_tensor(
                            wfin[:, fsl], wxy[:, fsl], wsig[2][:, fsl],
                            op=mybir.AluOpType.mult,
                        )
                        nc.sync.dma_start(
                            out_d[
                                g * 128 : (g + 1) * 128,
                                jw * WIDE + fsl.start : jw * WIDE + fsl.stop,
                            ],
                            wfin[:, fsl],
                        )
    _split_sync_waits(nc)
    return nc


_NC_CACHE = None


def _get_nc():
    global _NC_CACHE
    if _NC_CACHE is None:
        _NC_CACHE = _build_nc()
    return _NC_CACHE


def _host_prep(pos, dims, rot, points, valid_mask):
    pos = np.asarray(pos, dtype=np.float32)
    dims = np.asarray(dims, dtype=np.float32)
    rot = np.asarray(rot, dtype=np.float32)
    points = np.asarray(points, dtype=np.float32)
    valid_mask = np.asarray(valid_mask)

    pts = np.where(valid_mask[..., None], points, np.float32(0.0))  # (B,N,3)

    c = np.cos(rot[..., 0])  # (B,S)
    s = np.sin(rot[..., 0])
    tx, ty, tz = pos[..., 0], pos[..., 1], pos[..., 2]
    zero = np.zeros_like(c)
    # x,y rows of inv(s_T_box) against (px,py,1) homog coords
    rows = np.stack(
        [
            np.stack([c, s, -(c * tx + s * ty)], axis=-1),
            np.stack([-s, c, s * tx - c * ty], axis=-1),
        ],
        axis=-2,
    ).astype(np.float32)  # (B, S, 2, 3)

    # wall[k, (g,c)*128 + m]: block-diagonal over the two batches of g
    wall = np.zeros((6, NPAIR * 2 * 128), dtype=np.float32)
    for g in range(NPAIR):
        for cch in range(2):
            col0 = (g * 2 + cch) * 128
            for half in range(2):
                b = 2 * g + half
                # rows[b,:,cch,:]: (S, 3) -> k = 3*half..3*half+2
                wall[3 * half : 3 * half + 3, col0 + 64 * half : col0 + 64 * half + S] = (
                    rows[b, :, cch, :].T
                )

    hall = np.zeros((128, NPAIR * 3 + NPAIR), dtype=np.float32)
    harr = (SIGMOID_SLOPE * HALF * dims).astype(np.float32)  # (B,S,3)
    for g in range(NPAIR):
        for half in range(2):
            b = 2 * g + half
            hall[64 * half : 64 * half + S, g * 3 : g * 3 + 3] = harr[b]
            hall[64 * half : 64 * half + S, NPAIR * 3 + g] = tz[b]

    # rhs[g, k, n]: (px,py,1) of the two batches stacked along K (k=3*half)
    rhs = np.zeros((NPAIR, 6, N), dtype=np.float32)
    for g in range(NPAIR):
        for half in range(2):
            b = 2 * g + half
            rhs[g, 3 * half : 3 * half + 2] = pts[b, :, 0:2].T
            rhs[g, 3 * half + 2] = 1.0

    # pzb[g, p, n]: pz broadcast to the 128 box-rows of group g
    pzb = np.empty((NPAIR, 128, N), dtype=np.float32)
    for g in range(NPAIR):
        pzb[g, 0:64] = pts[2 * g, :, 2][None, :]
        pzb[g, 64:128] = pts[2 * g + 1, :, 2][None, :]
    return rhs, wall, hall, pzb


def kernel(pos, dims, rot, points, valid_mask, _want_trace=False):
    rhs, wall, hall, pzb = _host_prep(pos, dims, rot, points, valid_mask)

    in_maps = []
    for core in range(NCORES):
        n0 = core * NSH
        in_maps.append(
            {
                "rhs": np.ascontiguousarray(rhs[:, :, n0 : n0 + NSH]),
                "wall": wall,
                "hall": hall,
                "pzb": np.ascontiguousarray(pzb[:, :, n0 : n0 + NSH]),
            }
        )

    nc = _get_nc()
    res = run_bass_kernel_spmd(
        nc, in_maps, core_ids=list(range(NCORES)), trace=_want_trace
    )

    out = np.empty((B * S, N), dtype=np.float32)
    for core in range(NCORES):
        n0 = core * NSH
        out[:, n0 : n0 + NSH] = res.results[core]["out"].astype(np.float32)
    result = out.reshape(B, S, N)
    if _want_trace:
        return result, res
    return result
